# revision 70
# baseline (speedup 1.0000x reference)
"""Trainium2 Bass kernel for nn_AttentionLayer (B=4, S=2048, D=1024, H=16).

Self-contained: builds and compiles an SPMD Bass/Tile program once, then
runs it across 8 NeuronCores via run_bass_kernel_spmd.

Sharding (no collectives): core c handles batch b = c // 2 and query-token
half c % 2 (1024 query tokens). Each core receives pre-transposed fp8
activations (x^T slices) plus fp8/bf16 weights, computes its [1024, 1024]
slice of the final layernorm output in fp32, and the host reassembles.

v3 pipeline:
- K/Q^T/V projections and the FC context matmuls run in fp8 (e4m3) with
  DoubleRow perf mode: operands are staged as [128, 2, N] tiles holding
  two 128-row contraction slabs, halving matmul streaming time. Weights
  are host-prescaled by 64 (fp8 range); evacuations rescale by 1/64.
- Attention per head pair: scores^T = Kh @ Qh^T in bf16 with both heads'
  K=64 matmuls packed into one PSUM tile per query chunk — shared WAR
  deps make the scheduler emit them adjacently, so they row-tile onto
  disjoint PE-array halves and run concurrently.
- exp head A on ScalarE (native, bf16 out); head B on VectorE via a
  Schraudolph bit trick (one tensor_scalar fp32->int16 whose bits are
  bf16 exp values; the ~3% sawtooth error cancels in softmax).
- attn@V in bf16 with a per-head ones column producing denominators.
- Softmax normalization deferred: denominators roundtrip DRAM (bf16),
  reciprocal via a magic-number bit trick fused with the x64 fp8 scale,
  producing normalized fp8 context tiles in DoubleRow pair layout.
- FC: fp8 DR ctx matmuls + residual via PE identity matmuls (identity
  prescaled x4096) + bfc via a K=1 ones matmul; layernorm's rstd uses a
  Schraudolph bitcast-log2 -> Exp(ln2*x) seed + one Newton step so the
  whole kernel stays in the single exp ACT table set (no table switches).

v4 structure (ordered for overlap):
- All projection inputs load up-front on BOTH hwdge queues (sync+scalar)
  as few large 3D-rearranged DMAs (issue slots, not bandwidth, are the
  scarce resource); vx reuses the kx SBUF via region-level WAR deps.
- K proj is c0-outer so the first kx half feeds all 8 e-blocks; evacs
  alternate ScalarE/VectorE.
- Attention runs th-half-outer. The previous half's FC+layernorm blocks
  are emitted between the next half's units, so FC matmuls fill PE slack
  while exp paces attention, and FC borrows scores-ring PSUM slots
  (releasing them via a quick xs evacuation).
- NOTE: dma_start_transpose (xbar) for the residual raced on hardware
  (flaky wrong results on ~1/3 of runs) and walrus ignores
  InstMatmult.ldweights=False; both paths abandoned (RESID_XBAR=False).
"""

import numpy as np
import ml_dtypes


from contextlib import ExitStack

import concourse.bass as bass
import concourse.tile as tile
import concourse.mybir as mybir
from concourse import bacc

F32 = mybir.dt.float32
BF16 = mybir.dt.bfloat16
I16 = mybir.dt.int16
I8 = mybir.dt.int8
F8 = mybir.dt.float8e4
DR = mybir.MatmulPerfMode.DoubleRow
AF = mybir.ActivationFunctionType
ALU = mybir.AluOpType

LOG2E = 1.4426950408889634
# exp(x/8) ~= bf16_bits(int16(x * SCH_A + SCH_B)) (Schraudolph, bf16 top bits)
SCH_A = float((1 << 23) * LOG2E) * 0.125 / 65536.0
SCH_B = (float(127 << 23) - 366393.0) / 65536.0
# same trick to fp8e4m3 bits directly (int8 out): exp(x/8) ~= f8_bits(i8)
SCH_A8 = SCH_A / 16.0
SCH_B8 = (SCH_B - 15360.0) / 16.0
# 1/x ~= bf16_bits(RCP_MAGIC - bf16_bits(x)), x > 0 (max rel err ~5%)
RCP_MAGIC = 0x7EF3
# Schraudolph log2: log2(v) ~= bits_i32(v)/2^23 - (127 - 0.0436775)
LOG_A = -0.5 / 8388608.0
LOG_B = 0.5 * (127.0 - 0.0436775)
LN2 = 0.6931471805599453
RESID_XBAR = False  # xbar-DMA residual vs PE identity matmuls
WSCALE = 64.0          # host fp8 weight prescale
LAM = WSCALE * WSCALE  # fc psum scale (ctx*64 @ Wfc*64); LN is scale-invariant


def bcast_ap(ap: bass.AP, parts: int) -> bass.AP:
    """Partition-broadcast a [1, N]-shaped DRAM AP to [parts, N]."""
    return bass.AP(tensor=ap.tensor, offset=ap.offset,
                   ap=[[0, parts]] + list(ap.ap[-1:]))


def nsplits(total, cap=512):
    return [(i, min(cap, total - i)) for i in range(0, total, cap)]


def build(T=1024, S=2048, D=1024, H=16, DK=64, n_cores=8, eps=1e-5,
          trn_type="TRN2", apply_affine=True, apply_bfc=True,
          apply_bv=True):
    assert DK == 64 and H % 2 == 0 and D == H * DK
    G = D // 256      # DoubleRow contraction groups (256 rows each)
    EB = D // 128     # e blocks (projection output chunks); == H//2
    TB = T // 128
    SB = S // 128
    PAIRS = H // 2
    VW = 65           # per-head vp stripe: 64 v columns + 1 ones column

    nc = bacc.Bacc(trn_type, target_bir_lowering=False, debug=False,
                   num_devices=n_cores)

    qT = nc.dram_tensor("qT", [D, T], BF16, kind="ExternalInput").ap()
    kT = nc.dram_tensor("kT", [D, S], F8, kind="ExternalInput").ap()
    vT = nc.dram_tensor("vT", [D, S], F8, kind="ExternalInput").ap()
    Wq = nc.dram_tensor("Wq", [D, D], BF16, kind="ExternalInput").ap()
    Wk = nc.dram_tensor("Wk", [D, D], F8, kind="ExternalInput").ap()
    Wv = nc.dram_tensor("Wv", [D, D], F8, kind="ExternalInput").ap()
    Wfc = nc.dram_tensor("Wfc", [D, D], F8, kind="ExternalInput").ap()
    bq = nc.dram_tensor("bq", [D], F32, kind="ExternalInput").ap()
    bk = nc.dram_tensor("bk", [D], F32, kind="ExternalInput").ap()
    bv = nc.dram_tensor("bv", [D], F32, kind="ExternalInput").ap()
    bfch = nc.dram_tensor("bfch", [D], BF16, kind="ExternalInput").ap()
    gamma = nc.dram_tensor("gamma", [D], F32, kind="ExternalInput").ap()
    beta = nc.dram_tensor("beta", [D], F32, kind="ExternalInput").ap()
    ident = nc.dram_tensor("ident", [128, 128], BF16, kind="ExternalInput").ap()
    out = nc.dram_tensor("out", [T, D], F32, kind="ExternalOutput").ap()

    den_dram = nc.dram_tensor("den_scratch", [H, T], BF16).ap()

    def load_dr(pool, src, n, tagp, chunk=None, eng=None):
        """Load fp8 [D, n] DRAM tensor into G [128, 2, n] DoubleRow tiles.

        One DMA per (group, column-chunk) via a 3D rearranged source AP —
        DMA-queue issue slots (~0.6us each) are the scarce resource at
        kernel start, not bandwidth."""
        eng = eng or nc.sync
        tiles = [pool.tile([128, 2, n], F8, tag=f"{tagp}{g}", name=f"{tagp}{g}")
                 for g in range(G)]
        for c0, cn in nsplits(n, chunk or n):
            for g in range(G):
                eng.dma_start(
                    out=tiles[g][:, :, c0:c0 + cn],
                    in_=src[g * 256:(g + 1) * 256, c0:c0 + cn].rearrange(
                        "(r p) c -> p r c", p=128))
        return tiles

    def mm(*a, reuse=False, **k):
        """matmul; reuse=True marks it non-self-loading (the PE keeps the
        previous stationary operand). Only valid when the previous matmul
        in program order used the same lhsT and nothing can be scheduled
        between them (same-readiness pairs); verified by the rel-err gate."""
        i = nc.tensor.matmul(*a, **k)
        if reuse:
            i.ins.ldweights = False
        return i

    with tile.TileContext(nc) as tc, ExitStack() as ctx:
        pconst = ctx.enter_context(tc.tile_pool(name="const", bufs=1))
        ppers = ctx.enter_context(tc.tile_pool(name="persist", bufs=1))

        # ---- tiny constants -------------------------------------------
        bqT = pconst.tile([128, EB], F32, tag="bqT", name="bqT")
        nc.sync.dma_start(out=bqT, in_=bq.rearrange("(e p) -> p e", p=128))
        bkT = pconst.tile([128, EB], F32, tag="bkT", name="bkT")
        nc.sync.dma_start(out=bkT, in_=bk.rearrange("(e p) -> p e", p=128))
        i_sb = pconst.tile([128, 128], BF16, tag="ident", name="ident")
        nc.sync.dma_start(out=i_sb, in_=ident)
        ones1 = pconst.tile([1, 128], BF16, tag="ones1", name="ones1")
        nc.vector.memset(ones1, 1.0)
        bfc_sb = pconst.tile([1, D], BF16, tag="bfc_sb", name="bfc_sb")
        nc.sync.dma_start(out=bfc_sb, in_=bcast_ap(bfch, 1))

        # ---- persistent tiles -----------------------------------------
        kpT_sb = [ppers.tile([128, S], F8, tag=f"kpT{e}", name=f"kpT{e}")
                  for e in range(EB)]
        vp_dr = [ppers.tile([128, 2, H * VW], F8, tag=f"vp{s2}",
                            name=f"vp{s2}") for s2 in range(SB // 2)]
        ctxT_sb = [ppers.tile([128, T], BF16, tag=f"ctxT{e}", name=f"ctxT{e}")
                   for e in range(EB)]
        ctx8_sb = [ppers.tile([128, 2, T], F8, tag=f"ctx8_{g}",
                              name=f"ctx8_{g}") for g in range(G)]
        qpT_sb = [ppers.tile([128, T], BF16, tag=f"qpT{j}", name=f"qpT{j}")
                  for j in range(PAIRS)]
        # natural-layout qp (residual), filled by xbar DMA transposes
        qpnat = [ppers.tile([128, D], BF16, tag=f"qpnat{t}", name=f"qpnat{t}")
                 for t in range(TB)]

        # ===== projections: K (c0-outer), Q, V in one pool scope ========
        # DMA queue plan: sync = kx(g0,g1) -> qx -> vx(all, WAR-gated on
        # kx space); scalar = wk -> kx(g2,g3) -> wq -> wv (all fresh pools
        # so no issue-time blocking of the scalar engine's evacuations).
        with tc.tile_pool(name="wk", bufs=1) as pwk, \
             tc.tile_pool(name="kx", bufs=1) as pkx, \
             tc.tile_pool(name="qx", bufs=1) as pqx, \
             tc.tile_pool(name="wq", bufs=1) as pwq, \
             tc.tile_pool(name="wv", bufs=1) as pwv, \
             tc.tile_pool(name="vbc", bufs=1) as pvbc, \
             tc.tile_pool(name="kps", bufs=2, space="PSUM") as pps, \
             tc.tile_pool(name="qps", bufs=2, space="PSUM") as pqps:
            bv_bc = pvbc.tile([128, D], F32, tag="bv_bc", name="bv_bc")
            nc.gpsimd.dma_start(out=bv_bc, in_=bcast_ap(bv, 128))
            # split loads into ~256KB chunks: a single DMA only sustains
            # ~64GB/s, but chunks issued back-to-back transfer concurrently
            kx_dr = load_dr(pkx, kT, S, "kx", chunk=1024)
            wk_dr = load_dr(pwk, Wk, D, "wk", eng=nc.scalar)
            DB = D // 128
            qx_sb = pqx.tile([128, DB, T], BF16, tag="qx", name="qx")
            wq_sb = pwq.tile([128, DB, D], BF16, tag="wq", name="wq")
            for h in range(4):
                nc.sync.dma_start(
                    out=qx_sb[:, 2 * h:2 * h + 2, :],
                    in_=qT[h * 256:(h + 1) * 256, :].rearrange(
                        "(d p) t -> p d t", p=128))
                nc.scalar.dma_start(
                    out=wq_sb[:, 2 * h:2 * h + 2, :],
                    in_=Wq[h * 256:(h + 1) * 256, :].rearrange(
                        "(d p) t -> p d t", p=128))
            wv_dr = load_dr(pwv, Wv, D, "wv", eng=nc.scalar)
            # vx reuses the kx tiles (same tags -> same SBUF, WAR-ordered
            # behind the last K-proj readers of each column half).
            vx_dr = load_dr(pkx, vT, S, "kx", chunk=1024)

            CK = min(S, 1024)
            # c0-outer so the first half of the kx stream feeds all 8
            # e-blocks of matmuls before the second half must arrive.
            for c0, cn in nsplits(S, CK):
                for e in range(EB):
                    ecol = slice(e * 128, (e + 1) * 128)
                    ps = pps.tile([128, CK], F32, tag="kpT_ps", name="kpT_ps")
                    for g in range(G):
                        for n0, nn in nsplits(cn):
                            mm(ps[:, n0:n0 + nn],
                               lhsT=wk_dr[g][:, :, ecol],
                               rhs=kx_dr[g][:, :, c0 + n0:c0 + n0 + nn],
                               start=(g == 0), stop=(g == G - 1),
                               perf_mode=DR, reuse=(n0 > 0))
                    if e % 2 == 0:
                        nc.scalar.activation(
                            out=kpT_sb[e][:, c0:c0 + cn], in_=ps[:, 0:cn],
                            func=AF.Identity, scale=1.0 / WSCALE,
                            bias=bkT[:, e:e + 1])
                    else:
                        # alternate evac engine so Scalar/Vector split the
                        # PSUM drain and the projection pipeline never
                        # waits on a single engine
                        nc.vector.tensor_scalar(
                            out=kpT_sb[e][:, c0:c0 + cn], in0=ps[:, 0:cn],
                            scalar1=1.0 / WSCALE, scalar2=bkT[:, e:e + 1],
                            op0=ALU.mult, op1=ALU.add)

            # ============= Q^T projection (all pairs, bf16) ============
            for j in range(PAIRS):
                qps = pqps.tile([128, T], F32, tag="qps", name="qps")
                for d in range(DB):
                    for n0, nn in nsplits(T):
                        mm(qps[:, n0:n0 + nn],
                           lhsT=wq_sb[:, d, j * 128:(j + 1) * 128],
                           rhs=qx_sb[:, d, n0:n0 + nn],
                           start=(d == 0), stop=(d == DB - 1),
                           reuse=(n0 > 0))
                if j % 2 == 0:
                    nc.scalar.activation(out=qpT_sb[j], in_=qps,
                                         func=AF.Identity, scale=1.0,
                                         bias=bqT[:, j:j + 1])
                else:
                    nc.vector.tensor_scalar(
                        out=qpT_sb[j], in0=qps, scalar1=1.0,
                        scalar2=bqT[:, j:j + 1],
                        op0=ALU.mult, op1=ALU.add)

            # ========= V projection (natural layout, fp8 DR) ===========
            for s in range(SB):
                ps = pps.tile([128, D], F32, tag="kpT_ps", name="vp_ps")
                for g in range(G):
                    for n0, nn in nsplits(D):
                        mm(ps[:, n0:n0 + nn],
                           lhsT=vx_dr[g][:, :, s * 128:(s + 1) * 128],
                           rhs=wv_dr[g][:, :, n0:n0 + nn],
                           start=(g == 0), stop=(g == G - 1), perf_mode=DR,
                           reuse=(n0 > 0))
                vr = vp_dr[s // 2].rearrange("p k (h c) -> p k h c",
                                             c=VW)
                if apply_bv:
                    nc.vector.scalar_tensor_tensor(
                        out=vr[:, s % 2, :, 0:64],
                        in0=ps.rearrange("p (h c) -> p h c", c=DK),
                        scalar=1.0 / WSCALE,
                        in1=bv_bc.rearrange("p (h c) -> p h c", c=DK),
                        op0=ALU.mult, op1=ALU.add)
                elif s % 2 == 0:
                    nc.scalar.activation(
                        out=vr[:, s % 2, :, 0:64],
                        in_=ps.rearrange("p (h c) -> p h c", c=DK),
                        func=AF.Identity, scale=1.0 / WSCALE)
                else:
                    nc.vector.tensor_scalar(
                        out=vr[:, s % 2, :, 0:64],
                        in0=ps.rearrange("p (h c) -> p h c", c=DK),
                        scalar1=1.0 / WSCALE, scalar2=None, op0=ALU.mult)
                nc.vector.memset(vr[:, s % 2, :, 64:65], 1.0)

        # residual transposes via the DMA xbar (off the PE array), emitted
        # here so they queue BEHIND the projection input loads on the
        # sync/scalar DMA queues and drain during attention.
        if RESID_XBAR:
            for j in range(PAIRS):
                for t in range(TB):
                    eng = nc.sync if (j + t) % 2 == 0 else nc.scalar
                    eng.dma_start_transpose(
                        out=qpnat[t][:, j * 128:(j + 1) * 128],
                        in_=qpT_sb[j][:, t * 128:(t + 1) * 128])

        # ================= attention ====================================
        pwfc = ctx.enter_context(tc.tile_pool(name="wfc", bufs=1))
        wfc_dr = []
        for g in range(G):
            t = pwfc.tile([128, 2, D], F8, tag=f"wfc{g}", name=f"wfc{g}")
            nc.gpsimd.dma_start(
                out=t,
                in_=Wfc[g * 256:(g + 1) * 256, :].rearrange(
                    "(r p) c -> p r c", p=128))
            wfc_dr.append(t)

        with tc.tile_pool(name="scp", bufs=3, space="PSUM") as psc, \
             tc.tile_pool(name="cxps", bufs=1, space="PSUM") as pcx, \
             tc.tile_pool(name="atA", bufs=3) as pata, \
             tc.tile_pool(name="norm", bufs=2) as pnm, \
             tc.tile_pool(name="xln", bufs=2) as px, \
             tc.tile_pool(name="stat", bufs=4) as pst, \
             tc.tile_pool(name="lnbc", bufs=1) as plnb, \
             tc.tile_pool(name="ctmp", bufs=2) as ptmp:
            if apply_affine:
                gamma_bc = plnb.tile([128, D], F32, tag="gamma_bc",
                                     name="gamma_bc")
                nc.gpsimd.dma_start(out=gamma_bc, in_=bcast_ap(gamma, 128))
                beta_bc = plnb.tile([128, D], F32, tag="beta_bc",
                                    name="beta_bc")
                nc.gpsimd.dma_start(out=beta_bc, in_=bcast_ap(beta, 128))

            def fc_block(t):
                """FC + residual + layernorm for one 128-token block.

                Emitted right after the th-half that completes its ctx8
                inputs, so its matmuls fill TensorE stalls while the other
                half's attention (exp-paced) runs."""
                tblk = slice(t * 128, (t + 1) * 128)
                xs = px.tile([128, D], F32, tag="xs", name="xs")
                # fc borrows a scores-ring PSUM slot (same shape/tag) —
                # attention needs all 6 psc banks for MM<->exp pipelining,
                # and FC runs in the ring's natural WAR order; the xs evac
                # releases the slot quickly (holding it through the LN
                # chain starves the exp pipeline).
                fc = psc.tile([128, D], F32, tag="sc", name="fc")
                for g in range(G):
                    for c0, cn in nsplits(D):
                        mm(fc[:, c0:c0 + cn], lhsT=ctx8_sb[g][:, :, tblk],
                           rhs=wfc_dr[g][:, :, c0:c0 + cn],
                           start=(g == 0), stop=False,
                           perf_mode=DR, reuse=(c0 > 0))
                # residual via PE identity transposes of qpT
                for jj in range(TB):
                    nc.tensor.matmul(
                        fc[:, jj * 128:(jj + 1) * 128],
                        lhsT=qpT_sb[jj][:, tblk], rhs=i_sb,
                        start=False, stop=False)
                # bfc bias via K=1 ones matmuls (mark each bank's group end)
                for c0, cn in nsplits(D):
                    mm(fc[:, c0:c0 + cn], lhsT=ones1,
                       rhs=bfc_sb[0:1, c0:c0 + cn],
                       start=False, stop=True, reuse=(c0 > 0))
                nc.scalar.activation(out=xs, in_=fc, func=AF.Identity,
                                     scale=1.0 / LAM)
                ngr = max(D // 512, 1)
                gsz = min(D, 512)
                stats = pst.tile([128, ngr, 6], F32, tag="stats",
                                 name="stats")
                for g in range(ngr):
                    nc.vector.bn_stats(out=stats[:, g, :],
                                       in_=xs[:, g * gsz:(g + 1) * gsz])
                mv = pst.tile([128, 2], F32, tag="mv", name="mv")
                nc.vector.bn_aggr(out=mv, in_=stats)
                # rstd = (var+eps)^-0.5 without Sqrt/Ln (which would force
                # ACT table switches away from the exp set): Schraudolph
                # bitcast log2 -> Exp(ln2*x) = 2^x seed, then one Newton
                # step y*(1.5 - 0.5*v*y^2) for ~3e-4 rel err.
                ve = pst.tile([128, 1], F32, tag="ve", name="ve")
                nc.vector.tensor_scalar(out=ve, in0=mv[:, 1:2],
                                        scalar1=eps, scalar2=None,
                                        op0=ALU.add)
                lg = pst.tile([128, 1], F32, tag="lg", name="lg")
                nc.vector.tensor_scalar(out=lg,
                                        in0=ve.bitcast(mybir.dt.int32),
                                        scalar1=LOG_A, scalar2=LOG_B,
                                        op0=ALU.mult, op1=ALU.add)
                y0 = pst.tile([128, 1], F32, tag="y0", name="y0")
                nc.scalar.activation(out=y0, in_=lg, func=AF.Exp, scale=LN2)
                hf = pst.tile([128, 1], F32, tag="hf", name="hf")
                nc.vector.tensor_scalar(out=hf, in0=ve, scalar1=y0,
                                        scalar2=-0.5, op0=ALU.mult,
                                        op1=ALU.mult)
                nc.vector.tensor_scalar(out=hf, in0=hf, scalar1=y0,
                                        scalar2=None, op0=ALU.mult)
                rstd = pst.tile([128, 1], F32, tag="rstd", name="rstd")
                nc.vector.tensor_scalar(out=rstd, in0=hf, scalar1=1.5,
                                        scalar2=y0, op0=ALU.add,
                                        op1=ALU.mult)
                nmr = pst.tile([128, 1], F32, tag="nmr", name="nmr")
                nc.vector.tensor_scalar(out=nmr, in0=mv[:, 0:1],
                                        scalar1=rstd, scalar2=-1.0,
                                        op0=ALU.mult, op1=ALU.mult)
                xn = px.tile([128, D], F32, tag="xn", name="xn")
                if t % 2 == 0:
                    # gpsimd is the least-loaded engine and xs/xn are SBUF
                    nc.gpsimd.tensor_scalar(out=xn, in0=xs, scalar1=rstd,
                                            scalar2=nmr, op0=ALU.mult,
                                            op1=ALU.add)
                else:
                    # fp32 SBUF tensor_scalar runs 2x_1P on DVE (~0.6us)
                    nc.vector.tensor_scalar(out=xn, in0=xs, scalar1=rstd,
                                            scalar2=nmr, op0=ALU.mult,
                                            op1=ALU.add)
                if apply_affine:
                    xg = px.tile([128, D], F32, tag="xg", name="xg")
                    nc.vector.tensor_mul(out=xg, in0=xn, in1=gamma_bc)
                    nc.gpsimd.tensor_add(out=xg, in0=xg, in1=beta_bc)
                else:
                    xg = xn
                # A single 512KB output DMA sustains only ~60GB/s (~9us);
                # that's hidden under later work except for the last two
                # token blocks, whose output is chunked so the pieces
                # transfer concurrently at the kernel tail.
                if t >= TB - 2:
                    for c0, cn in nsplits(D, 512):
                        nc.sync.dma_start(out=out[tblk, c0:c0 + cn],
                                          in_=xg[:, c0:c0 + cn])
                else:
                    nc.sync.dma_start(out=out[tblk, :], in_=xg)

            HT = T // 2
            for th in range(2):
                tsl = slice(th * HT, (th + 1) * HT)
                for j in range(PAIRS):
                    # spread the previous half's FC blocks between this
                    # half's attention units: their matmuls fill PE slack
                    # while exp paces the units, and the first one lands
                    # after unit j=0 so the den->ctx8 chain latency of the
                    # previous half is hidden by attention work, not a
                    # PE-queue bubble.
                    if th == 1 and 1 <= j <= TB // 2:
                        fc_block(j - 1)
                    kA = kpT_sb[j][0:64, :]
                    kB = kpT_sb[j][64:128, :]
                    qA = qpT_sb[j][0:64, tsl]
                    qB = qpT_sb[j][64:128, tsl]
                    cxa = pcx.tile([VW, HT], F32, tag="cxA", name="cxA")
                    cxb = pcx.tile([VW, HT], F32, tag="cxB", name="cxB")
                    at_q = []

                    def attnv(m, cxa=cxa, cxb=cxb, j=j, at_q=at_q):
                        at8 = at_q[m]
                        vrA = vp_dr[m][:, :, 2 * j * VW:2 * j * VW + VW]
                        vrB = vp_dr[m][:, :,
                                       (2 * j + 1) * VW:(2 * j + 2) * VW]
                        st, sp = (m == 0), (m == SB // 2 - 1)
                        nc.tensor.matmul(cxa, lhsT=vrA, rhs=at8[:, :, 0:HT],
                                         start=st, stop=sp, perf_mode=DR)
                        nc.tensor.matmul(cxb, lhsT=vrB, rhs=at8[:, :, HT:T],
                                         start=st, stop=sp, perf_mode=DR)

                    for kb in range(SB):
                        kblk = slice(kb * 128, (kb + 1) * 128)
                        # one score tile holds both heads' chunk (A then
                        # B): shared WAR deps keep the two K=64 matmuls
                        # adjacent, so they row-tile concurrently; the
                        # 3-deep pool breaks the scores->exp->scores
                        # serial chain.
                        sc = psc.tile([128, T], F32, tag="sc", name="sc")
                        nc.tensor.matmul(sc[:, 0:HT], lhsT=kA[:, kblk],
                                         rhs=qA, start=True, stop=True)
                        nc.tensor.matmul(sc[:, HT:T], lhsT=kB[:, kblk],
                                         rhs=qB, start=True, stop=True)
                        # exp of the whole tile in ONE call, alternating
                        # engines per kb: ScalarE native exp (fp8 out) on
                        # even kb, VectorE Schraudolph-to-fp8-bits on odd.
                        sl = kb % 2
                        if sl == 0:
                            at8 = pata.tile([128, 2, T], F8, tag="at8",
                                            name="at8")
                            at_q.append(at8)
                            nc.scalar.activation(out=at8[:, 0, :], in_=sc,
                                                 func=AF.Exp, scale=0.125)
                        else:
                            nc.vector.tensor_scalar(
                                out=at8.bitcast(I8)[:, 1, :], in0=sc,
                                scalar1=SCH_A8, scalar2=SCH_B8,
                                op0=ALU.mult, op1=ALU.add)
                        # attn@V (fp8 DoubleRow, 256-key contraction),
                        # lagged one kb-pair so its inputs are complete
                        # and it never stalls TensorE's in-order queue.
                        if sl == 1 and kb // 2 >= 1:
                            attnv(kb // 2 - 1)
                    attnv(SB // 2 - 1)
                    # evacuate ctx + denominators: head A via DVE, head B
                    # via ScalarE (the PSUM-capable engines); den rows
                    # ride along in the [65, HT] staging copies.
                    stga = ptmp.tile([VW, HT], BF16, tag="stga", name="stga")
                    nc.scalar.activation(out=stga, in_=cxa, func=AF.Copy)
                    stgb = ptmp.tile([VW, HT], BF16, tag="stgb", name="stgb")
                    nc.scalar.activation(out=stgb, in_=cxb, func=AF.Copy)
                    nc.sync.dma_start(out=ctxT_sb[j][0:64, tsl],
                                      in_=stga[0:64, :])
                    nc.sync.dma_start(out=ctxT_sb[j][64:128, tsl],
                                      in_=stgb[0:64, :])
                    nc.gpsimd.dma_start(out=den_dram[2 * j, tsl],
                                        in_=stga[64:65, :])
                    nc.gpsimd.dma_start(out=den_dram[2 * j + 1, tsl],
                                        in_=stgb[64:65, :])
                    # deferred softmax normalization: magic-number bf16
                    # reciprocal of broadcast denominators, the x64 fp8
                    # ctx scale folded into the magic constant.
                    dbc = pnm.tile([128, HT], BF16, tag="dbc", name="dbc")
                    nc.gpsimd.dma_start(
                        out=dbc[0:64, :],
                        in_=bcast_ap(den_dram[2 * j:2 * j + 1, tsl], 64))
                    nc.gpsimd.dma_start(
                        out=dbc[64:128, :],
                        in_=bcast_ap(den_dram[2 * j + 1:2 * j + 2, tsl], 64))
                    rbc = pnm.tile([128, HT], I16, tag="rbc", name="rbc")
                    nc.gpsimd.tensor_scalar(out=rbc, in0=dbc.bitcast(I16),
                                            scalar1=-1,
                                            scalar2=RCP_MAGIC + (6 << 7),
                                            op0=ALU.mult, op1=ALU.add)
                    nc.gpsimd.tensor_mul(out=ctx8_sb[j // 2][:, j % 2, tsl],
                                         in0=ctxT_sb[j][:, tsl],
                                         in1=rbc.bitcast(BF16))
                if th == 1:
                    for t in range(TB // 2, TB):
                        fc_block(t)

    nc.compile()
    return nc


_B, _S, _D, _H, _DK = 4, 2048, 1024, 16, 64
_T = _S // 2
_NCORES = 8
_BF = ml_dtypes.bfloat16
_F8 = ml_dtypes.float8_e4m3

_nc_cache = {}


def _get_nc(apply_affine, apply_bfc, apply_bv=True):
    key = (apply_affine, apply_bfc, apply_bv)
    if key not in _nc_cache:
        _nc_cache[key] = build(T=_T, S=_S, D=_D, H=_H, DK=_DK,
                               n_cores=_NCORES, apply_affine=apply_affine,
                               apply_bfc=apply_bfc, apply_bv=apply_bv)
    return _nc_cache[key]


def _f8(x):
    return np.clip(x, -240.0, 240.0).astype(_F8)


def _execute(inputs, trace=False):
    from concourse.bass_utils import run_bass_kernel_spmd

    gamma_h = np.asarray(inputs["gamma"], np.float32)
    beta_h = np.asarray(inputs["beta"], np.float32)
    aff = not (np.all(gamma_h == 1.0) and np.all(beta_h == 0.0))
    bfc_h = np.asarray(inputs["bfc"], np.float32)
    bv_h = np.asarray(inputs["bv"], np.float32)
    nc = _get_nc(aff, bool(np.any(bfc_h != 0.0)), bool(np.any(bv_h != 0.0)))
    q = np.asarray(inputs["q"], np.float32)
    k = np.asarray(inputs["k"], np.float32)
    v = np.asarray(inputs["v"], np.float32)
    Wq = np.asarray(inputs["Wq"], np.float32).astype(_BF)
    Wk = _f8(np.asarray(inputs["Wk"], np.float32) * 64.0)
    Wv = _f8(np.asarray(inputs["Wv"], np.float32) * 64.0)
    Wfc = _f8(np.asarray(inputs["Wfc"], np.float32) * 64.0)
    fp = {n: np.asarray(inputs[n], np.float32)
          for n in ("bq", "bk", "bv", "gamma", "beta")}
    bfch = (np.asarray(inputs["bfc"], np.float32) * 4096.0).astype(_BF)
    ident = (np.eye(128, dtype=np.float32) * 4096.0).astype(_BF)

    in_maps = []
    for c in range(_NCORES):
        b, half = divmod(c, 2)
        t0 = half * _T
        in_maps.append({
            "qT": np.ascontiguousarray(q[b, t0:t0 + _T].T).astype(_BF),
            "kT": _f8(np.ascontiguousarray(k[b].T)),
            "vT": _f8(np.ascontiguousarray(v[b].T)),
            "Wq": Wq, "Wk": Wk, "Wv": Wv, "Wfc": Wfc,
            "bfch": bfch, "ident": ident, **fp,
        })

    res = run_bass_kernel_spmd(nc, in_maps, core_ids=list(range(_NCORES)),
                               trace=trace)
    out = np.empty((_B, _S, _D), np.float32)
    for c in range(_NCORES):
        b, half = divmod(c, 2)
        out[b, half * _T:(half + 1) * _T] = res.results[c]["out"]
    return out, res.exec_time_ns


def kernel(**inputs) -> np.ndarray:
    out, _ = _execute(inputs, trace=False)
    return out



# revision 71
# speedup vs baseline: 1.0224x; 1.0224x over previous
"""Trainium2 Bass kernel for nn_AttentionLayer (B=4, S=2048, D=1024, H=16).

Self-contained: builds and compiles an SPMD Bass/Tile program once, then
runs it across 8 NeuronCores via run_bass_kernel_spmd.

Sharding (no collectives): core c handles batch b = c // 2 and query-token
half c % 2 (1024 query tokens). Each core receives pre-transposed fp8
activations (x^T slices) plus fp8/bf16 weights, computes its [1024, 1024]
slice of the final layernorm output in fp32, and the host reassembles.

v3 pipeline:
- K/Q^T/V projections and the FC context matmuls run in fp8 (e4m3) with
  DoubleRow perf mode: operands are staged as [128, 2, N] tiles holding
  two 128-row contraction slabs, halving matmul streaming time. Weights
  are host-prescaled by 64 (fp8 range); evacuations rescale by 1/64.
- Attention per head pair: scores^T = Kh @ Qh^T in bf16 with both heads'
  K=64 matmuls packed into one PSUM tile per query chunk — shared WAR
  deps make the scheduler emit them adjacently, so they row-tile onto
  disjoint PE-array halves and run concurrently.
- exp head A on ScalarE (native, bf16 out); head B on VectorE via a
  Schraudolph bit trick (one tensor_scalar fp32->int16 whose bits are
  bf16 exp values; the ~3% sawtooth error cancels in softmax).
- attn@V in bf16 with a per-head ones column producing denominators.
- Softmax normalization deferred: denominators roundtrip DRAM (bf16),
  reciprocal via a magic-number bit trick fused with the x64 fp8 scale,
  producing normalized fp8 context tiles in DoubleRow pair layout.
- FC: fp8 DR ctx matmuls + residual via PE identity matmuls (identity
  prescaled x4096) + bfc via a K=1 ones matmul; layernorm's rstd uses a
  Schraudolph bitcast-log2 -> Exp(ln2*x) seed + one Newton step so the
  whole kernel stays in the single exp ACT table set (no table switches).

v4 structure (ordered for overlap):
- All projection inputs load up-front on BOTH hwdge queues (sync+scalar)
  as few large 3D-rearranged DMAs (issue slots, not bandwidth, are the
  scarce resource); vx reuses the kx SBUF via region-level WAR deps.
- K proj is c0-outer so the first kx half feeds all 8 e-blocks; evacs
  alternate ScalarE/VectorE.
- Attention runs th-half-outer. The previous half's FC+layernorm blocks
  are emitted between the next half's units, so FC matmuls fill PE slack
  while exp paces attention, and FC borrows scores-ring PSUM slots
  (releasing them via a quick xs evacuation).
- NOTE: dma_start_transpose (xbar) for the residual raced on hardware
  (flaky wrong results on ~1/3 of runs) and walrus ignores
  InstMatmult.ldweights=False; both paths abandoned (RESID_XBAR=False).
"""

import numpy as np
import ml_dtypes


from contextlib import ExitStack

import concourse.bass as bass
import concourse.tile as tile
import concourse.mybir as mybir
from concourse import bacc

F32 = mybir.dt.float32
BF16 = mybir.dt.bfloat16
I16 = mybir.dt.int16
I8 = mybir.dt.int8
F8 = mybir.dt.float8e4
DR = mybir.MatmulPerfMode.DoubleRow
AF = mybir.ActivationFunctionType
ALU = mybir.AluOpType

LOG2E = 1.4426950408889634
# exp(x/8) ~= bf16_bits(int16(x * SCH_A + SCH_B)) (Schraudolph, bf16 top bits)
SCH_A = float((1 << 23) * LOG2E) * 0.125 / 65536.0
SCH_B = (float(127 << 23) - 366393.0) / 65536.0
# same trick to fp8e4m3 bits directly (int8 out): exp(x/8) ~= f8_bits(i8)
SCH_A8 = SCH_A / 16.0
SCH_B8 = (SCH_B - 15360.0) / 16.0
# 1/x ~= bf16_bits(RCP_MAGIC - bf16_bits(x)), x > 0 (max rel err ~5%)
RCP_MAGIC = 0x7EF3
# Schraudolph log2: log2(v) ~= bits_i32(v)/2^23 - (127 - 0.0436775)
LOG_A = -0.5 / 8388608.0
LOG_B = 0.5 * (127.0 - 0.0436775)
LN2 = 0.6931471805599453
RESID_XBAR = False  # xbar-DMA residual vs PE identity matmuls
WSCALE = 64.0          # host fp8 weight prescale
LAM = WSCALE * WSCALE  # fc psum scale (ctx*64 @ Wfc*64); LN is scale-invariant


def bcast_ap(ap: bass.AP, parts: int) -> bass.AP:
    """Partition-broadcast a [1, N]-shaped DRAM AP to [parts, N]."""
    return bass.AP(tensor=ap.tensor, offset=ap.offset,
                   ap=[[0, parts]] + list(ap.ap[-1:]))


def nsplits(total, cap=512):
    return [(i, min(cap, total - i)) for i in range(0, total, cap)]


def build(T=1024, S=2048, D=1024, H=16, DK=64, n_cores=8, eps=1e-5,
          trn_type="TRN2", apply_affine=True, apply_bfc=True,
          apply_bv=True):
    assert DK == 64 and H % 2 == 0 and D == H * DK
    G = D // 256      # DoubleRow contraction groups (256 rows each)
    EB = D // 128     # e blocks (projection output chunks); == H//2
    TB = T // 128
    SB = S // 128
    PAIRS = H // 2
    VW = 65           # per-head vp stripe: 64 v columns + 1 ones column

    nc = bacc.Bacc(trn_type, target_bir_lowering=False, debug=False,
                   num_devices=n_cores)

    qT = nc.dram_tensor("qT", [D, T], BF16, kind="ExternalInput").ap()
    kT = nc.dram_tensor("kT", [D, S], F8, kind="ExternalInput").ap()
    vT = nc.dram_tensor("vT", [D, S], F8, kind="ExternalInput").ap()
    Wq = nc.dram_tensor("Wq", [D, D], BF16, kind="ExternalInput").ap()
    Wk = nc.dram_tensor("Wk", [D, D], F8, kind="ExternalInput").ap()
    Wv = nc.dram_tensor("Wv", [D, D], F8, kind="ExternalInput").ap()
    Wfc = nc.dram_tensor("Wfc", [D, D], F8, kind="ExternalInput").ap()
    bq = nc.dram_tensor("bq", [D], F32, kind="ExternalInput").ap()
    bk = nc.dram_tensor("bk", [D], F32, kind="ExternalInput").ap()
    bv = nc.dram_tensor("bv", [D], F32, kind="ExternalInput").ap()
    bfch = nc.dram_tensor("bfch", [D], BF16, kind="ExternalInput").ap()
    gamma = nc.dram_tensor("gamma", [D], F32, kind="ExternalInput").ap()
    beta = nc.dram_tensor("beta", [D], F32, kind="ExternalInput").ap()
    ident = nc.dram_tensor("ident", [128, 128], BF16, kind="ExternalInput").ap()
    out = nc.dram_tensor("out", [T, D], F32, kind="ExternalOutput").ap()

    den_dram = nc.dram_tensor("den_scratch", [H, T], BF16).ap()

    def load_dr(pool, src, n, tagp, chunk=None, eng=None):
        """Load fp8 [D, n] DRAM tensor into G [128, 2, n] DoubleRow tiles.

        One DMA per (group, column-chunk) via a 3D rearranged source AP —
        DMA-queue issue slots (~0.6us each) are the scarce resource at
        kernel start, not bandwidth."""
        eng = eng or nc.sync
        tiles = [pool.tile([128, 2, n], F8, tag=f"{tagp}{g}", name=f"{tagp}{g}")
                 for g in range(G)]
        for c0, cn in nsplits(n, chunk or n):
            for g in range(G):
                eng.dma_start(
                    out=tiles[g][:, :, c0:c0 + cn],
                    in_=src[g * 256:(g + 1) * 256, c0:c0 + cn].rearrange(
                        "(r p) c -> p r c", p=128))
        return tiles

    def mm(*a, reuse=False, **k):
        """matmul; reuse=True marks it non-self-loading (the PE keeps the
        previous stationary operand). Only valid when the previous matmul
        in program order used the same lhsT and nothing can be scheduled
        between them (same-readiness pairs); verified by the rel-err gate."""
        i = nc.tensor.matmul(*a, **k)
        if reuse:
            i.ins.ldweights = False
        return i

    with tile.TileContext(nc) as tc, ExitStack() as ctx:
        pconst = ctx.enter_context(tc.tile_pool(name="const", bufs=1))
        ppers = ctx.enter_context(tc.tile_pool(name="persist", bufs=1))

        # ---- tiny constants -------------------------------------------
        bqT = pconst.tile([128, EB], F32, tag="bqT", name="bqT")
        nc.sync.dma_start(out=bqT, in_=bq.rearrange("(e p) -> p e", p=128))
        bkT = pconst.tile([128, EB], F32, tag="bkT", name="bkT")
        nc.sync.dma_start(out=bkT, in_=bk.rearrange("(e p) -> p e", p=128))
        i_sb = pconst.tile([128, 128], BF16, tag="ident", name="ident")
        nc.sync.dma_start(out=i_sb, in_=ident)
        ones1 = pconst.tile([1, 128], BF16, tag="ones1", name="ones1")
        nc.vector.memset(ones1, 1.0)
        bfc_sb = pconst.tile([1, D], BF16, tag="bfc_sb", name="bfc_sb")
        nc.sync.dma_start(out=bfc_sb, in_=bcast_ap(bfch, 1))

        # ---- persistent tiles -----------------------------------------
        kpT_sb = [ppers.tile([128, S], F8, tag=f"kpT{e}", name=f"kpT{e}")
                  for e in range(EB)]
        vp_dr = [ppers.tile([128, 2, H * VW], F8, tag=f"vp{s2}",
                            name=f"vp{s2}") for s2 in range(SB // 2)]
        ctxT_sb = [ppers.tile([128, T], BF16, tag=f"ctxT{e}", name=f"ctxT{e}")
                   for e in range(EB)]
        ctx8_sb = [ppers.tile([128, 2, T], F8, tag=f"ctx8_{g}",
                              name=f"ctx8_{g}") for g in range(G)]
        qpT_sb = [ppers.tile([128, T], BF16, tag=f"qpT{j}", name=f"qpT{j}")
                  for j in range(PAIRS)]
        # natural-layout qp (residual), filled by xbar DMA transposes
        qpnat = [ppers.tile([128, D], BF16, tag=f"qpnat{t}", name=f"qpnat{t}")
                 for t in range(TB)]

        # ===== projections: K (c0-outer), Q, V in one pool scope ========
        # DMA queue plan: sync = kx(g0,g1) -> qx -> vx(all, WAR-gated on
        # kx space); scalar = wk -> kx(g2,g3) -> wq -> wv (all fresh pools
        # so no issue-time blocking of the scalar engine's evacuations).
        with tc.tile_pool(name="wk", bufs=1) as pwk, \
             tc.tile_pool(name="kx", bufs=1) as pkx, \
             tc.tile_pool(name="qx", bufs=1) as pqx, \
             tc.tile_pool(name="wq", bufs=1) as pwq, \
             tc.tile_pool(name="wv", bufs=1) as pwv, \
             tc.tile_pool(name="vbc", bufs=1) as pvbc, \
             tc.tile_pool(name="kps", bufs=2, space="PSUM") as pps, \
             tc.tile_pool(name="qps", bufs=2, space="PSUM") as pqps:
            bv_bc = pvbc.tile([128, D], F32, tag="bv_bc", name="bv_bc")
            nc.gpsimd.dma_start(out=bv_bc, in_=bcast_ap(bv, 128))
            # split loads into ~256KB chunks: a single DMA only sustains
            # ~64GB/s, but chunks issued back-to-back transfer concurrently
            kx_dr = load_dr(pkx, kT, S, "kx", chunk=1024)
            wk_dr = load_dr(pwk, Wk, D, "wk", eng=nc.scalar)
            DB = D // 128
            qx_sb = pqx.tile([128, DB, T], BF16, tag="qx", name="qx")
            wq_sb = pwq.tile([128, DB, D], BF16, tag="wq", name="wq")
            for h in range(4):
                nc.sync.dma_start(
                    out=qx_sb[:, 2 * h:2 * h + 2, :],
                    in_=qT[h * 256:(h + 1) * 256, :].rearrange(
                        "(d p) t -> p d t", p=128))
                nc.scalar.dma_start(
                    out=wq_sb[:, 2 * h:2 * h + 2, :],
                    in_=Wq[h * 256:(h + 1) * 256, :].rearrange(
                        "(d p) t -> p d t", p=128))
            wv_dr = load_dr(pwv, Wv, D, "wv", eng=nc.scalar)
            # vx reuses the kx tiles (same tags -> same SBUF, WAR-ordered
            # behind the last K-proj readers of each column half).
            vx_dr = load_dr(pkx, vT, S, "kx", chunk=1024)

            CK = min(S, 1024)
            # c0-outer so the first half of the kx stream feeds all 8
            # e-blocks of matmuls before the second half must arrive.
            for c0, cn in nsplits(S, CK):
                for e in range(EB):
                    ecol = slice(e * 128, (e + 1) * 128)
                    ps = pps.tile([128, CK], F32, tag="kpT_ps", name="kpT_ps")
                    for g in range(G):
                        for n0, nn in nsplits(cn):
                            mm(ps[:, n0:n0 + nn],
                               lhsT=wk_dr[g][:, :, ecol],
                               rhs=kx_dr[g][:, :, c0 + n0:c0 + n0 + nn],
                               start=(g == 0), stop=(g == G - 1),
                               perf_mode=DR, reuse=(n0 > 0))
                    if e % 2 == 0:
                        nc.scalar.activation(
                            out=kpT_sb[e][:, c0:c0 + cn], in_=ps[:, 0:cn],
                            func=AF.Identity, scale=1.0 / WSCALE,
                            bias=bkT[:, e:e + 1])
                    else:
                        # alternate evac engine so Scalar/Vector split the
                        # PSUM drain and the projection pipeline never
                        # waits on a single engine
                        nc.vector.tensor_scalar(
                            out=kpT_sb[e][:, c0:c0 + cn], in0=ps[:, 0:cn],
                            scalar1=1.0 / WSCALE, scalar2=bkT[:, e:e + 1],
                            op0=ALU.mult, op1=ALU.add)

            # ============= Q^T projection (all pairs, bf16) ============
            for j in range(PAIRS):
                qps = pqps.tile([128, T], F32, tag="qps", name="qps")
                for d in range(DB):
                    for n0, nn in nsplits(T):
                        mm(qps[:, n0:n0 + nn],
                           lhsT=wq_sb[:, d, j * 128:(j + 1) * 128],
                           rhs=qx_sb[:, d, n0:n0 + nn],
                           start=(d == 0), stop=(d == DB - 1),
                           reuse=(n0 > 0))
                if j % 2 == 0:
                    nc.scalar.activation(out=qpT_sb[j], in_=qps,
                                         func=AF.Identity, scale=1.0,
                                         bias=bqT[:, j:j + 1])
                else:
                    nc.vector.tensor_scalar(
                        out=qpT_sb[j], in0=qps, scalar1=1.0,
                        scalar2=bqT[:, j:j + 1],
                        op0=ALU.mult, op1=ALU.add)

            # ========= V projection (natural layout, fp8 DR) ===========
            for s in range(SB):
                ps = pps.tile([128, D], F32, tag="kpT_ps", name="vp_ps")
                for g in range(G):
                    for n0, nn in nsplits(D):
                        mm(ps[:, n0:n0 + nn],
                           lhsT=vx_dr[g][:, :, s * 128:(s + 1) * 128],
                           rhs=wv_dr[g][:, :, n0:n0 + nn],
                           start=(g == 0), stop=(g == G - 1), perf_mode=DR,
                           reuse=(n0 > 0))
                vr = vp_dr[s // 2].rearrange("p k (h c) -> p k h c",
                                             c=VW)
                if apply_bv:
                    nc.vector.scalar_tensor_tensor(
                        out=vr[:, s % 2, :, 0:64],
                        in0=ps.rearrange("p (h c) -> p h c", c=DK),
                        scalar=1.0 / WSCALE,
                        in1=bv_bc.rearrange("p (h c) -> p h c", c=DK),
                        op0=ALU.mult, op1=ALU.add)
                elif s % 2 == 0:
                    nc.scalar.activation(
                        out=vr[:, s % 2, :, 0:64],
                        in_=ps.rearrange("p (h c) -> p h c", c=DK),
                        func=AF.Identity, scale=1.0 / WSCALE)
                else:
                    nc.vector.tensor_scalar(
                        out=vr[:, s % 2, :, 0:64],
                        in0=ps.rearrange("p (h c) -> p h c", c=DK),
                        scalar1=1.0 / WSCALE, scalar2=None, op0=ALU.mult)
                nc.vector.memset(vr[:, s % 2, :, 64:65], 1.0)

        # residual transposes via the DMA xbar (off the PE array), emitted
        # here so they queue BEHIND the projection input loads on the
        # sync/scalar DMA queues and drain during attention.
        if RESID_XBAR:
            for j in range(PAIRS):
                for t in range(TB):
                    eng = nc.sync if (j + t) % 2 == 0 else nc.scalar
                    eng.dma_start_transpose(
                        out=qpnat[t][:, j * 128:(j + 1) * 128],
                        in_=qpT_sb[j][:, t * 128:(t + 1) * 128])

        # ================= attention ====================================
        pwfc = ctx.enter_context(tc.tile_pool(name="wfc", bufs=1))
        wfc_dr = []
        for g in range(G):
            t = pwfc.tile([128, 2, D], F8, tag=f"wfc{g}", name=f"wfc{g}")
            nc.gpsimd.dma_start(
                out=t,
                in_=Wfc[g * 256:(g + 1) * 256, :].rearrange(
                    "(r p) c -> p r c", p=128))
            wfc_dr.append(t)

        with tc.tile_pool(name="scp", bufs=3, space="PSUM") as psc, \
             tc.tile_pool(name="cxps", bufs=1, space="PSUM") as pcx, \
             tc.tile_pool(name="atA", bufs=3) as pata, \
             tc.tile_pool(name="norm", bufs=2) as pnm, \
             tc.tile_pool(name="xln", bufs=2) as px, \
             tc.tile_pool(name="stat", bufs=4) as pst, \
             tc.tile_pool(name="lnbc", bufs=1) as plnb, \
             tc.tile_pool(name="ctmp", bufs=2) as ptmp:
            if apply_affine:
                gamma_bc = plnb.tile([128, D], F32, tag="gamma_bc",
                                     name="gamma_bc")
                nc.gpsimd.dma_start(out=gamma_bc, in_=bcast_ap(gamma, 128))
                beta_bc = plnb.tile([128, D], F32, tag="beta_bc",
                                    name="beta_bc")
                nc.gpsimd.dma_start(out=beta_bc, in_=bcast_ap(beta, 128))

            def fc_block(t):
                """FC + residual + layernorm for one 128-token block.

                Emitted right after the th-half that completes its ctx8
                inputs, so its matmuls fill TensorE stalls while the other
                half's attention (exp-paced) runs."""
                tblk = slice(t * 128, (t + 1) * 128)
                xs = px.tile([128, D], F32, tag="xs", name="xs")
                # fc borrows a scores-ring PSUM slot (same shape/tag) —
                # attention needs all 6 psc banks for MM<->exp pipelining,
                # and FC runs in the ring's natural WAR order; the xs evac
                # releases the slot quickly (holding it through the LN
                # chain starves the exp pipeline).
                fc = psc.tile([128, D], F32, tag="sc", name="fc")
                for g in range(G):
                    for c0, cn in nsplits(D):
                        mm(fc[:, c0:c0 + cn], lhsT=ctx8_sb[g][:, :, tblk],
                           rhs=wfc_dr[g][:, :, c0:c0 + cn],
                           start=(g == 0), stop=False,
                           perf_mode=DR, reuse=(c0 > 0))
                # residual via PE identity transposes of qpT
                for jj in range(TB):
                    nc.tensor.matmul(
                        fc[:, jj * 128:(jj + 1) * 128],
                        lhsT=qpT_sb[jj][:, tblk], rhs=i_sb,
                        start=False, stop=False)
                # bfc bias via K=1 ones matmuls (mark each bank's group end)
                for c0, cn in nsplits(D):
                    mm(fc[:, c0:c0 + cn], lhsT=ones1,
                       rhs=bfc_sb[0:1, c0:c0 + cn],
                       start=False, stop=True, reuse=(c0 > 0))
                nc.scalar.activation(out=xs, in_=fc, func=AF.Identity,
                                     scale=1.0 / LAM)
                ngr = max(D // 512, 1)
                gsz = min(D, 512)
                stats = pst.tile([128, ngr, 6], F32, tag="stats",
                                 name="stats")
                for g in range(ngr):
                    nc.vector.bn_stats(out=stats[:, g, :],
                                       in_=xs[:, g * gsz:(g + 1) * gsz])
                mv = pst.tile([128, 2], F32, tag="mv", name="mv")
                nc.vector.bn_aggr(out=mv, in_=stats)
                # rstd = (var+eps)^-0.5 without Sqrt/Ln (which would force
                # ACT table switches away from the exp set): Schraudolph
                # bitcast log2 -> Exp(ln2*x) = 2^x seed, then one Newton
                # step y*(1.5 - 0.5*v*y^2) for ~3e-4 rel err.
                ve = pst.tile([128, 1], F32, tag="ve", name="ve")
                nc.vector.tensor_scalar(out=ve, in0=mv[:, 1:2],
                                        scalar1=eps, scalar2=None,
                                        op0=ALU.add)
                lg = pst.tile([128, 1], F32, tag="lg", name="lg")
                nc.vector.tensor_scalar(out=lg,
                                        in0=ve.bitcast(mybir.dt.int32),
                                        scalar1=LOG_A, scalar2=LOG_B,
                                        op0=ALU.mult, op1=ALU.add)
                y0 = pst.tile([128, 1], F32, tag="y0", name="y0")
                nc.scalar.activation(out=y0, in_=lg, func=AF.Exp, scale=LN2)
                hf = pst.tile([128, 1], F32, tag="hf", name="hf")
                nc.vector.tensor_scalar(out=hf, in0=ve, scalar1=y0,
                                        scalar2=-0.5, op0=ALU.mult,
                                        op1=ALU.mult)
                nc.vector.tensor_scalar(out=hf, in0=hf, scalar1=y0,
                                        scalar2=None, op0=ALU.mult)
                rstd = pst.tile([128, 1], F32, tag="rstd", name="rstd")
                nc.vector.tensor_scalar(out=rstd, in0=hf, scalar1=1.5,
                                        scalar2=y0, op0=ALU.add,
                                        op1=ALU.mult)
                nmr = pst.tile([128, 1], F32, tag="nmr", name="nmr")
                nc.vector.tensor_scalar(out=nmr, in0=mv[:, 0:1],
                                        scalar1=rstd, scalar2=-1.0,
                                        op0=ALU.mult, op1=ALU.mult)
                xn = px.tile([128, D], F32, tag="xn", name="xn")
                if t % 2 == 0:
                    # gpsimd is the least-loaded engine and xs/xn are SBUF
                    nc.gpsimd.tensor_scalar(out=xn, in0=xs, scalar1=rstd,
                                            scalar2=nmr, op0=ALU.mult,
                                            op1=ALU.add)
                else:
                    # fp32 SBUF tensor_scalar runs 2x_1P on DVE (~0.6us)
                    nc.vector.tensor_scalar(out=xn, in0=xs, scalar1=rstd,
                                            scalar2=nmr, op0=ALU.mult,
                                            op1=ALU.add)
                if apply_affine:
                    xg = px.tile([128, D], F32, tag="xg", name="xg")
                    nc.vector.tensor_mul(out=xg, in0=xn, in1=gamma_bc)
                    nc.gpsimd.tensor_add(out=xg, in0=xg, in1=beta_bc)
                else:
                    xg = xn
                # A single 512KB output DMA sustains only ~60GB/s (~9us);
                # that's hidden under later work except for the last two
                # token blocks, whose output is chunked so the pieces
                # transfer concurrently at the kernel tail.
                if t >= TB - 2:
                    for c0, cn in nsplits(D, 512):
                        nc.sync.dma_start(out=out[tblk, c0:c0 + cn],
                                          in_=xg[:, c0:c0 + cn])
                else:
                    nc.sync.dma_start(out=out[tblk, :], in_=xg)

            HT = T // 2
            for th in range(2):
                tsl = slice(th * HT, (th + 1) * HT)
                for j in range(PAIRS):
                    # spread the previous half's FC blocks between this
                    # half's attention units: their matmuls fill PE slack
                    # while exp paces the units, and the first one lands
                    # after unit j=0 so the den->ctx8 chain latency of the
                    # previous half is hidden by attention work, not a
                    # PE-queue bubble.
                    if th == 1 and 1 <= j <= 2:
                        fc_block(j - 1)
                    kA = kpT_sb[j][0:64, :]
                    kB = kpT_sb[j][64:128, :]
                    qA = qpT_sb[j][0:64, tsl]
                    qB = qpT_sb[j][64:128, tsl]
                    cxa = pcx.tile([VW, HT], F32, tag="cxA", name="cxA")
                    cxb = pcx.tile([VW, HT], F32, tag="cxB", name="cxB")
                    at_q = []

                    def attnv(m, cxa=cxa, cxb=cxb, j=j, at_q=at_q):
                        at8 = at_q[m]
                        vrA = vp_dr[m][:, :, 2 * j * VW:2 * j * VW + VW]
                        vrB = vp_dr[m][:, :,
                                       (2 * j + 1) * VW:(2 * j + 2) * VW]
                        st, sp = (m == 0), (m == SB // 2 - 1)
                        nc.tensor.matmul(cxa, lhsT=vrA, rhs=at8[:, :, 0:HT],
                                         start=st, stop=sp, perf_mode=DR)
                        nc.tensor.matmul(cxb, lhsT=vrB, rhs=at8[:, :, HT:T],
                                         start=st, stop=sp, perf_mode=DR)

                    for kb in range(SB):
                        kblk = slice(kb * 128, (kb + 1) * 128)
                        # one score tile holds both heads' chunk (A then
                        # B): shared WAR deps keep the two K=64 matmuls
                        # adjacent, so they row-tile concurrently; the
                        # 3-deep pool breaks the scores->exp->scores
                        # serial chain.
                        sc = psc.tile([128, T], F32, tag="sc", name="sc")
                        nc.tensor.matmul(sc[:, 0:HT], lhsT=kA[:, kblk],
                                         rhs=qA, start=True, stop=True)
                        nc.tensor.matmul(sc[:, HT:T], lhsT=kB[:, kblk],
                                         rhs=qB, start=True, stop=True)
                        # exp of the whole tile in ONE call, alternating
                        # engines per kb: ScalarE native exp (fp8 out) on
                        # even kb, VectorE Schraudolph-to-fp8-bits on odd.
                        sl = kb % 2
                        if sl == 0:
                            at8 = pata.tile([128, 2, T], F8, tag="at8",
                                            name="at8")
                            at_q.append(at8)
                            nc.scalar.activation(out=at8[:, 0, :], in_=sc,
                                                 func=AF.Exp, scale=0.125)
                        else:
                            nc.vector.tensor_scalar(
                                out=at8.bitcast(I8)[:, 1, :], in0=sc,
                                scalar1=SCH_A8, scalar2=SCH_B8,
                                op0=ALU.mult, op1=ALU.add)
                        # attn@V (fp8 DoubleRow, 256-key contraction),
                        # lagged one kb-pair so its inputs are complete
                        # and it never stalls TensorE's in-order queue.
                        if sl == 1 and kb // 2 >= 1:
                            attnv(kb // 2 - 1)
                    attnv(SB // 2 - 1)
                    # evacuate ctx + denominators: head A via DVE, head B
                    # via ScalarE (the PSUM-capable engines); den rows
                    # ride along in the [65, HT] staging copies.
                    stga = ptmp.tile([VW, HT], BF16, tag="stga", name="stga")
                    nc.scalar.activation(out=stga, in_=cxa, func=AF.Copy)
                    stgb = ptmp.tile([VW, HT], BF16, tag="stgb", name="stgb")
                    nc.scalar.activation(out=stgb, in_=cxb, func=AF.Copy)
                    nc.sync.dma_start(out=ctxT_sb[j][0:64, tsl],
                                      in_=stga[0:64, :])
                    nc.sync.dma_start(out=ctxT_sb[j][64:128, tsl],
                                      in_=stgb[0:64, :])
                    nc.gpsimd.dma_start(out=den_dram[2 * j, tsl],
                                        in_=stga[64:65, :])
                    nc.gpsimd.dma_start(out=den_dram[2 * j + 1, tsl],
                                        in_=stgb[64:65, :])
                    # deferred softmax normalization: magic-number bf16
                    # reciprocal of broadcast denominators, the x64 fp8
                    # ctx scale folded into the magic constant.
                    dbc = pnm.tile([128, HT], BF16, tag="dbc", name="dbc")
                    nc.gpsimd.dma_start(
                        out=dbc[0:64, :],
                        in_=bcast_ap(den_dram[2 * j:2 * j + 1, tsl], 64))
                    nc.gpsimd.dma_start(
                        out=dbc[64:128, :],
                        in_=bcast_ap(den_dram[2 * j + 1:2 * j + 2, tsl], 64))
                    rbc = pnm.tile([128, HT], I16, tag="rbc", name="rbc")
                    nc.gpsimd.tensor_scalar(out=rbc, in0=dbc.bitcast(I16),
                                            scalar1=-1,
                                            scalar2=RCP_MAGIC + (6 << 7),
                                            op0=ALU.mult, op1=ALU.add)
                    nc.gpsimd.tensor_mul(out=ctx8_sb[j // 2][:, j % 2, tsl],
                                         in0=ctxT_sb[j][:, tsl],
                                         in1=rbc.bitcast(BF16))
                if th == 1:
                    # blocks 2,3 (ready since th0) land here to keep the
                    # PE fed while the last unit's den chain resolves,
                    # then the th1 blocks follow.
                    for t in range(2, TB):
                        fc_block(t)

    nc.compile()
    return nc


_B, _S, _D, _H, _DK = 4, 2048, 1024, 16, 64
_T = _S // 2
_NCORES = 8
_BF = ml_dtypes.bfloat16
_F8 = ml_dtypes.float8_e4m3

_nc_cache = {}


def _get_nc(apply_affine, apply_bfc, apply_bv=True):
    key = (apply_affine, apply_bfc, apply_bv)
    if key not in _nc_cache:
        _nc_cache[key] = build(T=_T, S=_S, D=_D, H=_H, DK=_DK,
                               n_cores=_NCORES, apply_affine=apply_affine,
                               apply_bfc=apply_bfc, apply_bv=apply_bv)
    return _nc_cache[key]


def _f8(x):
    return np.clip(x, -240.0, 240.0).astype(_F8)


def _execute(inputs, trace=False):
    from concourse.bass_utils import run_bass_kernel_spmd

    gamma_h = np.asarray(inputs["gamma"], np.float32)
    beta_h = np.asarray(inputs["beta"], np.float32)
    aff = not (np.all(gamma_h == 1.0) and np.all(beta_h == 0.0))
    bfc_h = np.asarray(inputs["bfc"], np.float32)
    bv_h = np.asarray(inputs["bv"], np.float32)
    nc = _get_nc(aff, bool(np.any(bfc_h != 0.0)), bool(np.any(bv_h != 0.0)))
    q = np.asarray(inputs["q"], np.float32)
    k = np.asarray(inputs["k"], np.float32)
    v = np.asarray(inputs["v"], np.float32)
    Wq = np.asarray(inputs["Wq"], np.float32).astype(_BF)
    Wk = _f8(np.asarray(inputs["Wk"], np.float32) * 64.0)
    Wv = _f8(np.asarray(inputs["Wv"], np.float32) * 64.0)
    Wfc = _f8(np.asarray(inputs["Wfc"], np.float32) * 64.0)
    fp = {n: np.asarray(inputs[n], np.float32)
          for n in ("bq", "bk", "bv", "gamma", "beta")}
    bfch = (np.asarray(inputs["bfc"], np.float32) * 4096.0).astype(_BF)
    ident = (np.eye(128, dtype=np.float32) * 4096.0).astype(_BF)

    in_maps = []
    for c in range(_NCORES):
        b, half = divmod(c, 2)
        t0 = half * _T
        in_maps.append({
            "qT": np.ascontiguousarray(q[b, t0:t0 + _T].T).astype(_BF),
            "kT": _f8(np.ascontiguousarray(k[b].T)),
            "vT": _f8(np.ascontiguousarray(v[b].T)),
            "Wq": Wq, "Wk": Wk, "Wv": Wv, "Wfc": Wfc,
            "bfch": bfch, "ident": ident, **fp,
        })

    res = run_bass_kernel_spmd(nc, in_maps, core_ids=list(range(_NCORES)),
                               trace=trace)
    out = np.empty((_B, _S, _D), np.float32)
    for c in range(_NCORES):
        b, half = divmod(c, 2)
        out[b, half * _T:(half + 1) * _T] = res.results[c]["out"]
    return out, res.exec_time_ns


def kernel(**inputs) -> np.ndarray:
    out, _ = _execute(inputs, trace=False)
    return out

